# revision 9
# baseline (speedup 1.0000x reference)
"""Causal MHA with RoPE on 8 Trainium2 NeuronCores — v4.

Sharding: core c -> batch b = c//2, head-group g = c%2 (8 heads each).
Per core: qkv projection (bf16 matmuls, fp32 accum), RoPE (DVE, bf16 out),
SBUF->SBUF DMA re-layout of q/k into head-contiguous form (64 dims of one
head in 64 contiguous partitions) so each scores tile is a single K=64
matmul, causal attention in scores^T layout [k, q] (softmax without
max-subtraction, denominator via ones-row appended to V, causal mask as a
post-exp 0/1 multiply on GpSimd), output projection (partial over the
core's heads). Host sums the two head-group partials per batch + bias.

The qkv work for q-super t+1 is interleaved into the attention of super t;
proj(t-1) is likewise emitted inside att(t). PSUM is organized as 2-bank
[128,1024] tiles (score pairs share one exp), per-head softmax tail is
last-PV -> {reciprocal || pv-copy} -> bones-broadcast-matmul -> y-mul with
dedicated PSUM slots so consecutive heads never serialize on it.

Hardcoded problem: B=4, T=2048, C=1024, H=16, hs=64.
"""
import math
import numpy as np
import contextlib
import ml_dtypes

import concourse.bass as bass
import concourse.tile as tile
from concourse import bacc, mybir
from concourse.bass_utils import run_bass_kernel_spmd

B, T, C = 4, 2048, 1024
H, HS = 16, 64
HG = 8            # heads per core
N_CORES = 8
NQ = 512          # q-super width
NKT = T // 128    # 16 k-tiles
NJS = T // NQ     # 4 q-supers

f32 = mybir.dt.float32
f32r = mybir.dt.float32r
bf16 = mybir.dt.bfloat16
EXP = mybir.ActivationFunctionType.Exp
BF = ml_dtypes.bfloat16

_NC_CACHE = {}
LEVEL = 3


def _m2(hl):
    """free-slice index of head hl in the head-contiguous q/k layout"""
    return 2 * ((hl % 4) // 2) + hl // 4


def _p2(hl):
    """partition base of head hl in the head-contiguous q/k layout"""
    return 64 * (hl % 2)


def build_nc(iters: int = 1):
    key = (iters, LEVEL)
    if key in _NC_CACHE:
        return _NC_CACHE[key]
    nc = bacc.Bacc("TRN2", target_bir_lowering=False, debug=False,
                   num_devices=N_CORES)

    xt_ap = nc.dram_tensor("xt", [C, T], bf16, kind="ExternalInput").ap()
    wqkv_ap = nc.dram_tensor("wqkv", [C, 3, 512], bf16, kind="ExternalInput").ap()
    wp_ap = nc.dram_tensor("wp", [512, C], bf16, kind="ExternalInput").ap()
    cos_ap = nc.dram_tensor("cost", [128, T], f32, kind="ExternalInput").ap()
    sin_ap = nc.dram_tensor("sint", [128, T], f32, kind="ExternalInput").ap()
    msk_ap = nc.dram_tensor("mask01", [128, 128], bf16, kind="ExternalInput").ap()
    one_ap = nc.dram_tensor("vones", [128, NKT, HG, 1], bf16, kind="ExternalInput").ap()
    bones_ap = nc.dram_tensor("bones", [1, 64], f32r, kind="ExternalInput").ap()
    out_ap = nc.dram_tensor("outT", [C, T], f32, kind="ExternalOutput").ap()

    with tile.TileContext(nc) as tc, contextlib.ExitStack() as ctx:
        pq2 = ctx.enter_context(tc.tile_pool(name="pq2", bufs=1))
        pk2 = ctx.enter_context(tc.tile_pool(name="pk2", bufs=1))
        pvt = ctx.enter_context(tc.tile_pool(name="pvt", bufs=1))
        pstage = ctx.enter_context(tc.tile_pool(name="pstage", bufs=4))
        pxt = ctx.enter_context(tc.tile_pool(name="pxt", bufs=2))
        pw = ctx.enter_context(tc.tile_pool(name="pw", bufs=1))
        ptab = ctx.enter_context(tc.tile_pool(name="ptab", bufs=1))
        pscr = ctx.enter_context(tc.tile_pool(name="pscr", bufs=4))
        pexp = ctx.enter_context(tc.tile_pool(name="pexp", bufs=4))
        pyt = ctx.enter_context(tc.tile_pool(name="pyt", bufs=2))
        pyu = ctx.enter_context(tc.tile_pool(name="pyu", bufs=2))
        pmask = ctx.enter_context(tc.tile_pool(name="pmask", bufs=1))
        prc = ctx.enter_context(tc.tile_pool(name="prc", bufs=2))
        pout = ctx.enter_context(tc.tile_pool(name="pout", bufs=3))
        pbig = ctx.enter_context(tc.tile_pool(name="pbig", bufs=3, space="PSUM"))
        ppv = ctx.enter_context(tc.tile_pool(name="ppv", bufs=1, space="PSUM"))
        pbc = ctx.enter_context(tc.tile_pool(name="pbc", bufs=1, space="PSUM"))

        # constants loaded once (outside the timing loop)
        wqkv_t = pw.tile([128, 8, 3, 512], bf16, tag="wqkv")
        nc.sync.dma_start(out=wqkv_t,
                          in_=wqkv_ap.rearrange("(kt p) s n -> p kt s n", p=128))
        wp_t = pw.tile([128, 4, C], bf16, tag="wp")
        nc.sync.dma_start(out=wp_t, in_=wp_ap.rearrange("(kt p) e -> p kt e", p=128))
        cos_t = ptab.tile([128, T], f32, tag="cos")
        nc.sync.dma_start(out=cos_t, in_=cos_ap)
        sin_t = ptab.tile([128, T], f32, tag="sin")
        nc.sync.dma_start(out=sin_t, in_=sin_ap)
        mask_t = pmask.tile([128, 128], bf16, tag="mask")
        nc.sync.dma_start(out=mask_t, in_=msk_ap)
        bones_t = pmask.tile([1, 64], f32r, tag="bones")
        nc.sync.dma_start(out=bones_t, in_=bones_ap)

        def body(_iv):
            # persistent-per-iteration tensors (head-contiguous q/k, v)
            qt2_t = pq2.tile([128, 4, T], bf16, tag="qt2")
            kt2_t = pk2.tile([128, 4, T], bf16, tag="kt2")
            v_t = pvt.tile([128, NKT, HG, HS + 1], bf16, tag="vt")
            nc.sync.dma_start(out=v_t[:, :, :, HS:HS + 1], in_=one_ap)

            xts = {}
            stgs = {}

            def load_xt(ts):
                xt_t = pxt.tile([128, 8, NQ], bf16, tag="xt", name=f"xt{ts}")
                tsl = slice(ts * NQ, (ts + 1) * NQ)
                nc.sync.dma_start(
                    out=xt_t,
                    in_=xt_ap[:, tsl].rearrange("(kt p) n -> p kt n", p=128))
                xts[ts] = xt_t

            def qkv_group(ts, g):
                """g 0,1: q pair g; g 2,3: k pair g-2; g 4,5: v pair."""
                tsl = slice(ts * NQ, (ts + 1) * NQ)
                xt_t = xts[ts]
                big = pbig.tile([128, 2, NQ], f32, tag="big", name=f"qkv{ts}{g}")
                if g < 4:
                    s, pi = (0, g) if g < 2 else (1, g - 2)
                    if pi == 0:
                        stgs[(ts, s)] = pstage.tile(
                            [128, 4, NQ], bf16, tag="stg", name=f"stg{ts}{s}")
                    stg = stgs[(ts, s)]
                    pe = big[:, 0, :]
                    po = big[:, 1, :]
                    for kt in range(8):
                        nc.tensor.matmul(
                            pe, wqkv_t[:, kt, s, pi * 128:pi * 128 + 128],
                            xt_t[:, kt, :], start=(kt == 0), stop=(kt == 7))
                        nc.tensor.matmul(
                            po, wqkv_t[:, kt, s, (pi + 2) * 128:(pi + 3) * 128],
                            xt_t[:, kt, :], start=(kt == 0), stop=(kt == 7))
                    ct = cos_t[:, tsl]
                    st = sin_t[:, tsl]
                    t1 = pscr.tile([128, NQ], f32, tag="scr")
                    t2 = pscr.tile([128, NQ], f32, tag="scr")
                    nc.vector.tensor_mul(t1[:], pe, ct)
                    nc.vector.tensor_mul(t2[:], po, st)
                    nc.vector.tensor_sub(stg[:, pi, :], t1[:], t2[:])
                    t3 = pscr.tile([128, NQ], f32, tag="scr")
                    t4 = pscr.tile([128, NQ], f32, tag="scr")
                    nc.vector.tensor_mul(t3[:], pe, st)
                    nc.vector.tensor_mul(t4[:], po, ct)
                    nc.vector.tensor_add(stg[:, pi + 2, :], t3[:], t4[:])
                    if pi == 1:
                        # re-layout into head-contiguous tiles (SBUF->SBUF DMA)
                        dst = qt2_t if s == 0 else kt2_t
                        for a in range(4):
                            for par in range(2):
                                pb = 64 * (a % 2) + 32 * par
                                mb = 2 * (a // 2)
                                nc.sync.dma_start(
                                    out=dst[pb:pb + 32, mb:mb + 2, tsl],
                                    in_=stg[32 * a:32 * a + 32,
                                            2 * par:2 * par + 2, :])
                        del stgs[(ts, s)]
                else:
                    tb = 0 if g == 4 else 2
                    for half in (0, 1):
                        tt = tb + half
                        for kt in range(8):
                            nc.tensor.matmul(
                                big[:, half, :],
                                xt_t[:, kt, tt * 128:(tt + 1) * 128],
                                wqkv_t[:, kt, 2, :], start=(kt == 0), stop=(kt == 7))
                    nc.vector.tensor_copy(
                        v_t[:, ts * 4 + tb:ts * 4 + tb + 2, :, 0:HS], big[:])

            def att_head(j, hl, yt_t):
                m2 = _m2(hl)
                p2 = _p2(hl)
                pv_ps = ppv.tile([HS + 1, NQ], f32, tag="pv", name=f"pv{j}{hl}")
                nk = 4 * j + 4
                for u in range(nk // 2):
                    i0, i1 = 2 * u, 2 * u + 1
                    r0, r1 = i0 - 4 * j, i1 - 4 * j
                    big = pbig.tile([128, 2, NQ], f32, tag="big")
                    ex = pexp.tile([128, 2, NQ], bf16, tag="exp")
                    for idx, i, rr in ((0, i0, r0), (1, i1, r1)):
                        n0 = 0 if rr < 0 else 128 * rr
                        qv = slice(j * NQ + n0, (j + 1) * NQ)
                        isl = slice(i * 128, (i + 1) * 128)
                        nc.tensor.matmul(
                            big[:, idx, n0:], kt2_t[p2:p2 + 64, m2, isl],
                            qt2_t[p2:p2 + 64, m2, qv], start=True, stop=True)
                    if r1 < 0:
                        nc.scalar.activation(ex[:], big[:], EXP)
                    else:
                        for idx, rr in ((0, r0), (1, r1)):
                            n0 = 128 * rr
                            nc.scalar.activation(ex[:, idx, n0:],
                                                 big[:, idx, n0:], EXP)
                            nc.gpsimd.tensor_mul(ex[:, idx, n0:n0 + 128],
                                                 ex[:, idx, n0:n0 + 128],
                                                 mask_t[:])
                    for idx, i, rr in ((0, i0, r0), (1, i1, r1)):
                        n0 = 0 if rr < 0 else 128 * rr
                        nc.tensor.matmul(
                            pv_ps[:, n0:], v_t[:, i, hl, :], ex[:, idx, n0:],
                            start=(i == 0), stop=(i == nk - 1))
                if LEVEL < 3:
                    nc.vector.tensor_copy(
                        yt_t[(hl % 2) * 64:(hl % 2) * 64 + 64, hl // 2, :],
                        pv_ps[0:HS, :])
                    return
                yu = pyu.tile([64, NQ], f32, tag="yu")
                nc.vector.tensor_copy(yu[:], pv_ps[0:HS, :])
                rc = prc.tile([1, NQ], f32r, tag="rc")
                with nc.allow_low_precision(reason="f32r is 32-bit"):
                    nc.vector.reciprocal(rc[:], pv_ps[HS:HS + 1, :])
                bcp = pbc.tile([64, NQ], f32, tag="bc", name=f"bc{j}{hl}")
                nc.tensor.matmul(bcp[:], bones_t[:], rc[:], start=True, stop=True)
                nc.vector.tensor_mul(
                    yt_t[(hl % 2) * 64:(hl % 2) * 64 + 64, hl // 2, :],
                    yu[:], bcp[:])

            def proj(j, yt_t):
                jsl = slice(j * NQ, (j + 1) * NQ)
                for mp in range(4):
                    big = pbig.tile([128, 2, NQ], f32, tag="big")
                    for half in (0, 1):
                        m = 2 * mp + half
                        for kt in range(4):
                            nc.tensor.matmul(
                                big[:, half, :], wp_t[:, kt, m * 128:(m + 1) * 128],
                                yt_t[:, kt, :], start=(kt == 0), stop=(kt == 3))
                    ob = pout.tile([128, 2, NQ], f32, tag="ob")
                    nc.vector.tensor_copy(ob[:], big[:])
                    nc.sync.dma_start(
                        out=out_ap[2 * mp * 128:(2 * mp + 2) * 128, jsl]
                        .rearrange("(mm p) n -> p mm n", p=128),
                        in_=ob[:])

            # prologue: qkv for super 0
            load_xt(0)
            for g in range(6):
                qkv_group(0, g)

            if LEVEL < 2:
                for ts in range(1, NJS):
                    load_xt(ts)
                    for g in range(6):
                        qkv_group(ts, g)
                ob0 = pout.tile([128, 2, NQ], f32, tag="ob")
                nc.vector.tensor_copy(ob0[:, 0, :], qt2_t[:, 0, 0:NQ])
                nc.sync.dma_start(out=out_ap[0:128, 0:NQ], in_=ob0[:, 0, :])
                return

            yts = {}
            for t in range(NJS):
                if t < NJS - 1:
                    load_xt(t + 1)
                yts[t] = pyt.tile([128, 4, NQ], bf16, tag="yt", name=f"yt{t}")
                for hl in range(HG):
                    att_head(t, hl, yts[t])
                    if hl < 6 and t < NJS - 1:
                        qkv_group(t + 1, hl)
                    if hl == 6 and t > 0:
                        proj(t - 1, yts[t - 1])
            proj(NJS - 1, yts[NJS - 1])

        if iters == 1:
            body(0)
        else:
            with tc.For_i(0, iters) as iv:
                body(iv)

    nc.compile()
    _NC_CACHE[key] = nc
    return nc


def make_in_maps(x, W_qkv, W_proj):
    """Per-core host-side sharding + RoPE-layout permutation."""
    # x1-first column permutation within a head-group (8 heads x 64 dims):
    # [h0 evens, h1 evens, ..., h7 evens, h0 odds, ..., h7 odds]
    perm = []
    for parity in (0, 1):
        for hlc in range(HG):
            perm.extend(hlc * HS + d for d in range(parity, HS, 2))
    perm = np.asarray(perm)

    pos = np.arange(T, dtype=np.float64)
    inv_freq = 1.0 / (10000.0 ** (np.arange(0, HS, 2, dtype=np.float64) / HS))
    freqs = pos[:, None] * inv_freq[None, :]          # (T, 32)
    cost = np.tile(np.cos(freqs).T, (4, 1)).astype(np.float32)   # (128, T)
    sint = np.tile(np.sin(freqs).T, (4, 1)).astype(np.float32)

    kk = np.arange(128)[:, None]
    qq = np.arange(128)[None, :]
    mask01 = (kk <= qq).astype(BF)                    # (128, 128) 0/1
    vones = np.ones((128, NKT, HG, 1), BF)

    scale = 1.0 / math.sqrt(HS)
    in_maps = []
    for c in range(N_CORES):
        b, g = c // 2, c % 2
        base = g * HG * HS
        wq = W_qkv[:, base + perm] * scale
        wk = W_qkv[:, C + base + perm]
        wv = W_qkv[:, 2 * C + base: 2 * C + base + HG * HS]
        wqkv = np.stack([wq, wk, wv], axis=1).astype(BF)  # (C, 3, 512)
        in_maps.append({
            "xt": np.ascontiguousarray(x[b].T).astype(BF),
            "wqkv": np.ascontiguousarray(wqkv),
            "wp": np.ascontiguousarray(W_proj[base:base + HG * HS, :]).astype(BF),
            "cost": cost, "sint": sint, "mask01": mask01, "vones": vones,
            "bones": np.ones((1, 64), np.float32),
        })
    return in_maps


def kernel(x, W_qkv, W_proj, b_proj):
    x = np.asarray(x); W_qkv = np.asarray(W_qkv)
    W_proj = np.asarray(W_proj); b_proj = np.asarray(b_proj)
    nc = build_nc(1)
    in_maps = make_in_maps(x, W_qkv, W_proj)
    res = run_bass_kernel_spmd(nc, in_maps, list(range(N_CORES)))
    out = np.empty((B, T, C), np.float32)
    for b in range(B):
        acc = res.results[2 * b]["outT"] + res.results[2 * b + 1]["outT"]
        out[b] = acc.T + b_proj[None, :]
    return out


# revision 24
# speedup vs baseline: 1.5682x; 1.5682x over previous
"""Causal MHA with RoPE on 8 Trainium2 NeuronCores — v4.

Sharding: core c -> batch b = c//2, head-group g = c%2 (8 heads each).
Per core: qkv projection (bf16 matmuls, fp32 accum), RoPE (DVE, bf16 out),
SBUF->SBUF DMA re-layout of q/k into head-contiguous form (64 dims of one
head in 64 contiguous partitions) so each scores tile is a single K=64
matmul, causal attention in scores^T layout [k, q] (softmax without
max-subtraction, denominator via ones-row appended to V, causal mask as a
post-exp 0/1 multiply on GpSimd), output projection (partial over the
core's heads). Host sums the two head-group partials per batch + bias.

The qkv work for q-super t+1 is interleaved into the attention of super t;
proj(t-1) is likewise emitted inside att(t). PSUM is organized as 2-bank
[128,1024] tiles (score pairs share one exp), per-head softmax tail is
last-PV -> {reciprocal || pv-copy} -> bones-broadcast-matmul -> y-mul with
dedicated PSUM slots so consecutive heads never serialize on it.

Hardcoded problem: B=4, T=2048, C=1024, H=16, hs=64.
"""
import math
import numpy as np
import contextlib
import ml_dtypes

import concourse.bass as bass
import concourse.tile as tile
from concourse import bacc, mybir
from concourse.bass_utils import run_bass_kernel_spmd

B, T, C = 4, 2048, 1024
H, HS = 16, 64
HG = 8            # heads per core
N_CORES = 8
NQ = 512          # q-super width
NKT = T // 128    # 16 k-tiles
NJS = T // NQ     # 4 q-supers

f32 = mybir.dt.float32
f32r = mybir.dt.float32r
bf16 = mybir.dt.bfloat16
EXP = mybir.ActivationFunctionType.Exp
BF = ml_dtypes.bfloat16

_NC_CACHE = {}
LEVEL = 3


def _m2(hl):
    """free-slice index of head hl in the head-contiguous q/k layout"""
    return 2 * ((hl % 4) // 2) + hl // 4


def _p2(hl):
    """partition base of head hl in the head-contiguous q/k layout"""
    return 64 * (hl % 2)


def build_nc(iters: int = 1):
    key = (iters, LEVEL)
    if key in _NC_CACHE:
        return _NC_CACHE[key]
    nc = bacc.Bacc("TRN2", target_bir_lowering=False, debug=False,
                   num_devices=N_CORES)

    xt_ap = nc.dram_tensor("xt", [C, T], bf16, kind="ExternalInput").ap()
    wqkv_ap = nc.dram_tensor("wqkv", [C, 3, 512], bf16, kind="ExternalInput").ap()
    wp_ap = nc.dram_tensor("wp", [512, C], bf16, kind="ExternalInput").ap()
    cos_ap = nc.dram_tensor("cost", [128, T], f32, kind="ExternalInput").ap()
    sin_ap = nc.dram_tensor("sint", [128, T], f32, kind="ExternalInput").ap()
    msk_ap = nc.dram_tensor("mask01", [128, 128], bf16, kind="ExternalInput").ap()
    one_ap = nc.dram_tensor("vones", [128, NKT, HG, 64], bf16, kind="ExternalInput").ap()
    out_ap = nc.dram_tensor("outT", [C, T], f32, kind="ExternalOutput").ap()

    with tile.TileContext(nc) as tc, contextlib.ExitStack() as ctx:
        pq2 = ctx.enter_context(tc.tile_pool(name="pq2", bufs=1))
        pk2 = ctx.enter_context(tc.tile_pool(name="pk2", bufs=1))
        pvt = ctx.enter_context(tc.tile_pool(name="pvt", bufs=1))
        pstage = ctx.enter_context(tc.tile_pool(name="pstage", bufs=4))
        pxt = ctx.enter_context(tc.tile_pool(name="pxt", bufs=2))
        pw = ctx.enter_context(tc.tile_pool(name="pw", bufs=1))
        ptab = ctx.enter_context(tc.tile_pool(name="ptab", bufs=1))
        pscr = ctx.enter_context(tc.tile_pool(name="pscr", bufs=4))
        pexp = ctx.enter_context(tc.tile_pool(name="pexp", bufs=6))
        pyt = ctx.enter_context(tc.tile_pool(name="pyt", bufs=2))
        pmask = ctx.enter_context(tc.tile_pool(name="pmask", bufs=1))
        prc = ctx.enter_context(tc.tile_pool(name="prc", bufs=2))
        pout = ctx.enter_context(tc.tile_pool(name="pout", bufs=3))
        pbig = ctx.enter_context(tc.tile_pool(name="pbig", bufs=3, space="PSUM"))
        ppv = ctx.enter_context(tc.tile_pool(name="ppv", bufs=2, space="PSUM"))

        # constants loaded once (outside the timing loop)
        wqkv_t = pw.tile([128, 8, 3, 512], bf16, tag="wqkv")
        nc.sync.dma_start(out=wqkv_t,
                          in_=wqkv_ap.rearrange("(kt p) s n -> p kt s n", p=128))
        wp_t = pw.tile([128, 4, C], bf16, tag="wp")
        nc.sync.dma_start(out=wp_t, in_=wp_ap.rearrange("(kt p) e -> p kt e", p=128))
        cos_t = ptab.tile([128, T], f32, tag="cos")
        nc.sync.dma_start(out=cos_t, in_=cos_ap)
        sin_t = ptab.tile([128, T], f32, tag="sin")
        nc.sync.dma_start(out=sin_t, in_=sin_ap)
        mask_t = pmask.tile([128, 128], bf16, tag="mask")
        nc.sync.dma_start(out=mask_t, in_=msk_ap)
        v_t = pvt.tile([128, NKT, HG, HS + 64], bf16, tag="vt")
        nc.sync.dma_start(out=v_t[:, :, :, HS:], in_=one_ap)

        def body(_iv):
            # persistent-per-iteration tensors (head-contiguous q/k)
            qt2_t = pq2.tile([128, 4, T], bf16, tag="qt2")
            kt2_t = pk2.tile([128, 4, T], bf16, tag="kt2")

            xts = {}
            stgs = {}

            def load_xt(ts):
                xt_t = pxt.tile([128, 8, NQ], bf16, tag="xt", name=f"xt{ts}")
                tsl = slice(ts * NQ, (ts + 1) * NQ)
                nc.sync.dma_start(
                    out=xt_t,
                    in_=xt_ap[:, tsl].rearrange("(kt p) n -> p kt n", p=128))
                xts[ts] = xt_t

            def qkv_group(ts, g):
                """g 0,1: q pair g; g 2,3: k pair g-2; g 4,5: v pair."""
                tsl = slice(ts * NQ, (ts + 1) * NQ)
                xt_t = xts[ts]
                big = pbig.tile([128, 2, NQ], f32, tag="big", name=f"qkv{ts}{g}")
                if g < 4:
                    s, pi = (0, g) if g < 2 else (1, g - 2)
                    if pi == 0:
                        stgs[(ts, s)] = pstage.tile(
                            [128, 4, NQ], bf16, tag="stg", name=f"stg{ts}{s}")
                    stg = stgs[(ts, s)]
                    pe = big[:, 0, :]
                    po = big[:, 1, :]
                    for kt in range(8):
                        nc.tensor.matmul(
                            pe, wqkv_t[:, kt, s, pi * 128:pi * 128 + 128],
                            xt_t[:, kt, :], start=(kt == 0), stop=(kt == 7))
                        nc.tensor.matmul(
                            po, wqkv_t[:, kt, s, (pi + 2) * 128:(pi + 3) * 128],
                            xt_t[:, kt, :], start=(kt == 0), stop=(kt == 7))
                    ct = cos_t[:, tsl]
                    st = sin_t[:, tsl]
                    t1 = pscr.tile([128, NQ], f32, tag="scr")
                    t2 = pscr.tile([128, NQ], f32, tag="scr")
                    nc.vector.tensor_mul(t1[:], pe, ct)
                    nc.vector.tensor_mul(t2[:], po, st)
                    nc.vector.tensor_sub(stg[:, pi, :], t1[:], t2[:])
                    t3 = pscr.tile([128, NQ], f32, tag="scr")
                    t4 = pscr.tile([128, NQ], f32, tag="scr")
                    nc.vector.tensor_mul(t3[:], pe, st)
                    nc.vector.tensor_mul(t4[:], po, ct)
                    nc.vector.tensor_add(stg[:, pi + 2, :], t3[:], t4[:])
                    if pi == 1:
                        # re-layout into head-contiguous tiles (SBUF->SBUF DMA)
                        dst = qt2_t if s == 0 else kt2_t
                        for a in range(4):
                            for par in range(2):
                                pb = 64 * (a % 2) + 32 * par
                                mb = 2 * (a // 2)
                                nc.sync.dma_start(
                                    out=dst[pb:pb + 32, mb:mb + 2, tsl],
                                    in_=stg[32 * a:32 * a + 32,
                                            2 * par:2 * par + 2, :])
                        del stgs[(ts, s)]
                else:
                    tb = 0 if g == 4 else 2
                    for half in (0, 1):
                        tt = tb + half
                        for kt in range(8):
                            nc.tensor.matmul(
                                big[:, half, :],
                                xt_t[:, kt, tt * 128:(tt + 1) * 128],
                                wqkv_t[:, kt, 2, :], start=(kt == 0), stop=(kt == 7))
                    nc.vector.tensor_copy(
                        v_t[:, ts * 4 + tb:ts * 4 + tb + 2, :, 0:HS], big[:])

            def att_head(j, hl, yt_t):
                m2 = _m2(hl)
                p2 = _p2(hl)
                pv_ps = ppv.tile([128, NQ], f32, tag="pv", name=f"pv{j}{hl}")
                nk = 4 * j + 4
                for u in range(nk // 2):
                    i0, i1 = 2 * u, 2 * u + 1
                    r0, r1 = i0 - 4 * j, i1 - 4 * j
                    big = pbig.tile([128, 2, NQ], f32, tag="big")
                    ex = pexp.tile([128, 2, NQ], bf16, tag="exp")
                    for idx, i, rr in ((0, i0, r0), (1, i1, r1)):
                        n0 = 0 if rr < 0 else 128 * rr
                        qv = slice(j * NQ + n0, (j + 1) * NQ)
                        isl = slice(i * 128, (i + 1) * 128)
                        nc.tensor.matmul(
                            big[:, idx, n0:], kt2_t[p2:p2 + 64, m2, isl],
                            qt2_t[p2:p2 + 64, m2, qv], start=True, stop=True)
                    if r1 < 0:
                        nc.scalar.activation(ex[:], big[:], EXP)
                    else:
                        for idx, rr in ((0, r0), (1, r1)):
                            n0 = 128 * rr
                            nc.scalar.activation(ex[:, idx, n0:],
                                                 big[:, idx, n0:], EXP)
                            nc.gpsimd.tensor_mul(ex[:, idx, n0:n0 + 128],
                                                 ex[:, idx, n0:n0 + 128],
                                                 mask_t[:])
                    for idx, i, rr in ((0, i0, r0), (1, i1, r1)):
                        n0 = 0 if rr < 0 else 128 * rr
                        nc.tensor.matmul(
                            pv_ps[:, n0:], v_t[:, i, hl, :], ex[:, idx, n0:],
                            start=(i == 0), stop=(i == nk - 1))
                if LEVEL < 3:
                    nc.vector.tensor_copy(
                        yt_t[(hl % 2) * 64:(hl % 2) * 64 + 64, hl // 2, :],
                        pv_ps[0:HS, :])
                    return
                dcp = prc.tile([64, NQ], f32, tag="dc")
                nc.vector.tensor_copy(dcp[:], pv_ps[HS:, :])
                rcb = prc.tile([64, NQ], f32, tag="rc")
                nc.vector.reciprocal_approx_fast(out=rcb[:], in_=dcp[:])
                nc.vector.tensor_mul(
                    yt_t[(hl % 2) * 64:(hl % 2) * 64 + 64, hl // 2, :],
                    pv_ps[0:HS, :], rcb[:])

            def proj_piece(j, yt_t, mp, ob_on_act):
                jsl = slice(j * NQ, (j + 1) * NQ)
                big = pbig.tile([128, 2, NQ], f32, tag="big")
                for half in (0, 1):
                    m = 2 * mp + half
                    for kt in range(4):
                        nc.tensor.matmul(
                            big[:, half, :], wp_t[:, kt, m * 128:(m + 1) * 128],
                            yt_t[:, kt, :], start=(kt == 0), stop=(kt == 3))
                ob = pout.tile([128, 2, NQ], f32, tag="ob")
                if ob_on_act:
                    nc.scalar.copy(ob[:], big[:])
                else:
                    nc.vector.tensor_copy(ob[:], big[:])
                nc.sync.dma_start(
                    out=out_ap[2 * mp * 128:(2 * mp + 2) * 128, jsl]
                    .rearrange("(mm p) n -> p mm n", p=128),
                    in_=ob[:])

            def proj(j, yt_t):
                for mp in range(4):
                    proj_piece(j, yt_t, mp, False)

            # prologue: qkv for super 0
            load_xt(0)
            for g in range(6):
                qkv_group(0, g)

            if LEVEL < 2:
                for ts in range(1, NJS):
                    load_xt(ts)
                    for g in range(6):
                        qkv_group(ts, g)
                ob0 = pout.tile([128, 2, NQ], f32, tag="ob")
                nc.vector.tensor_copy(ob0[:, 0, :], qt2_t[:, 0, 0:NQ])
                nc.sync.dma_start(out=out_ap[0:128, 0:NQ], in_=ob0[:, 0, :])
                return

            yts = {}
            for t in range(NJS):
                if t < NJS - 1:
                    load_xt(t + 1)
                yts[t] = pyt.tile([128, 4, NQ], bf16, tag="yt", name=f"yt{t}")
                for hl in range(HG):
                    att_head(t, hl, yts[t])
                    if hl < 6 and t < NJS - 1:
                        qkv_group(t + 1, hl)
                    if hl >= 4 and t > 0:
                        proj_piece(t - 1, yts[t - 1], hl - 4, t - 1 <= 1)
            proj(NJS - 1, yts[NJS - 1])

        if iters == 1:
            body(0)
        else:
            with tc.For_i(0, iters,
                          hint_engines=(mybir.EngineType.PE,
                                        mybir.EngineType.DVE,
                                        mybir.EngineType.Activation,
                                        mybir.EngineType.SP,
                                        mybir.EngineType.Pool)) as iv:
                body(iv)

    nc.compile()
    _NC_CACHE[key] = nc
    return nc


def make_in_maps(x, W_qkv, W_proj):
    """Per-core host-side sharding + RoPE-layout permutation."""
    # x1-first column permutation within a head-group (8 heads x 64 dims):
    # [h0 evens, h1 evens, ..., h7 evens, h0 odds, ..., h7 odds]
    perm = []
    for parity in (0, 1):
        for hlc in range(HG):
            perm.extend(hlc * HS + d for d in range(parity, HS, 2))
    perm = np.asarray(perm)

    pos = np.arange(T, dtype=np.float64)
    inv_freq = 1.0 / (10000.0 ** (np.arange(0, HS, 2, dtype=np.float64) / HS))
    freqs = pos[:, None] * inv_freq[None, :]          # (T, 32)
    cost = np.tile(np.cos(freqs).T, (4, 1)).astype(np.float32)   # (128, T)
    sint = np.tile(np.sin(freqs).T, (4, 1)).astype(np.float32)

    kk = np.arange(128)[:, None]
    qq = np.arange(128)[None, :]
    mask01 = (kk <= qq).astype(BF)                    # (128, 128) 0/1
    vones = np.ones((128, NKT, HG, 64), BF)

    scale = 1.0 / math.sqrt(HS)
    in_maps = []
    for c in range(N_CORES):
        b, g = c // 2, c % 2
        base = g * HG * HS
        wq = W_qkv[:, base + perm] * scale
        wk = W_qkv[:, C + base + perm]
        wv = W_qkv[:, 2 * C + base: 2 * C + base + HG * HS]
        wqkv = np.stack([wq, wk, wv], axis=1).astype(BF)  # (C, 3, 512)
        in_maps.append({
            "xt": np.ascontiguousarray(x[b].T).astype(BF),
            "wqkv": np.ascontiguousarray(wqkv),
            "wp": np.ascontiguousarray(W_proj[base:base + HG * HS, :]).astype(BF),
            "cost": cost, "sint": sint, "mask01": mask01, "vones": vones,
        })
    return in_maps


def kernel(x, W_qkv, W_proj, b_proj):
    x = np.asarray(x); W_qkv = np.asarray(W_qkv)
    W_proj = np.asarray(W_proj); b_proj = np.asarray(b_proj)
    nc = build_nc(1)
    in_maps = make_in_maps(x, W_qkv, W_proj)
    res = run_bass_kernel_spmd(nc, in_maps, list(range(N_CORES)))
    out = np.empty((B, T, C), np.float32)
    for b in range(B):
        acc = res.results[2 * b]["outT"] + res.results[2 * b + 1]["outT"]
        out[b] = acc.T + b_proj[None, :]
    return out
